# revision 12
# baseline (speedup 1.0000x reference)
"""Trainium2 Bass kernel for nn_Differ_Amplifier (gnn_message_passing).

Reference computation (per layer i, h0 = x [N, H]):
    represent = (N*h - colsum(h)) / (N-1)
    h = represent @ W_i.T + h
    out_i = sigmoid(h @ W_ff.T + b_ff)

Reformulation (exact algebra, validated vs fp64):
  - colsum(h) is invariant across layers (the centered "represent" sums
    to zero), so total = colsum(x), computed on the HOST from the full
    input - no collective needed at all.
  - Composing the per-layer affine maps on the host:
        h_{i+1} = h_i @ V_i - r_i,   V_i = I + c*W_i^T,  c = N/(N-1)
        M_{i+1} = M_i @ V_i,         s_{i+1} = s_i @ V_i + r_i
        out_i   = sigmoid(x @ G_i + c_i),
        G_i = M_{i+1} @ W_ff^T,      c_i = b_ff - s_{i+1} @ W_ff^T
    Four independent [rows,512]@[512,512] matmuls; the bias is a
    per-output-column constant.

Device schedule (per core, rows = 4096, everything fp16 except PSUM):
  - x is uploaded pre-transposed (x^T, fp16) so no on-device transpose.
  - Output is computed TRANSPOSED: out^T tiles [128 o-part, rows free].
    lhsT (stationary) = G blocks [128 h, 128 o], moving = x^T slices
    [128 h, 512 rows]. This makes the bias c_i[o] a per-PARTITION
    scalar, so the ACT engine applies sigmoid(z + bias) in a single op
    straight out of PSUM -> fp16 SBUF. No DVE work at all.
  - PE runs one uninterrupted stream of 512 N=512 fp16 matmuls
    (~213ns each at full clock); PSUM rotates 8 banks in two half-sets
    so ACT eviction of one half overlaps matmuls of the other.
  - DMA queues: sync=x^T in, gpsimd=weights in, vector=out^T out.
    All transfers are large and linear; host reassembles/casts fp32.
"""

import numpy as np

import concourse.bass as bass  # noqa: F401
import concourse.tile as tile
from concourse import bacc, mybir
from concourse import bass_utils

N_CORES = 8
N_TOTAL = 32768
H = 512
OUT = 512
L = 4
P = 128
KC = H // P    # 4 k-chunks of the hidden (contraction) dim
OC = OUT // P  # 4 output-column chunks
F16 = mybir.dt.float16
F32 = mybir.dt.float32
SIG = mybir.ActivationFunctionType.Sigmoid


def _row_chunks(rbt):
    """Split rbt row-blocks (512 rows each) into chunks.

    First and last chunks are single blocks (fast pipeline start, short
    tail); the middle is split into near-equal chunks of <= 4 blocks
    (one PSUM half-set each).
    """
    if rbt <= 2:
        sizes = [1] * rbt
    else:
        rem = rbt - 2
        parts = -(-rem // 4)
        base, extra = divmod(rem, parts)
        sizes = [1] + [base + (1 if j < extra else 0) for j in range(parts)] + [1]
    chunks = []
    rb = 0
    for n in sizes:
        chunks.append((rb, n))
        rb += n
    return chunks


def build(rows=N_TOTAL // N_CORES):
    """Build the SPMD kernel for one core owning `rows` rows."""
    assert rows % 512 == 0
    RBT = rows // 512
    chunks = _row_chunks(RBT)
    NCH = len(chunks)

    nc = bacc.Bacc(
        "TRN2", target_bir_lowering=False, debug=False, num_devices=N_CORES
    )
    # x^T fp16, packed chunk-major: for ci: for k: block [P, n*512]
    # raveled, so every DMA is fully linear
    xt = nc.dram_tensor("xt", [KC * P * rows], F16,
                        kind="ExternalInput").ap()
    # G blocks fp16 per layer: [P(h), (k*OC+oc)*P + m]
    gt = nc.dram_tensor("gt", [L, P, KC * OC * P], F16,
                        kind="ExternalInput").ap()
    # bias per-partition scalars: cb[p, i*OC+oc] = c_i[oc*P+p]
    cb = nc.dram_tensor("cb", [P, L * OC], F32, kind="ExternalInput").ap()
    # transposed output: [L, OC, P(o), rows]
    outT = nc.dram_tensor("outT", [L, OC, P, rows], F16,
                          kind="ExternalOutput").ap()

    with tile.TileContext(nc) as tc:
        with (
            tc.tile_pool(name="wpool", bufs=1) as wpool,
            tc.tile_pool(name="xpool", bufs=1) as xpool,
            tc.tile_pool(name="opool", bufs=1) as opool,
            tc.tile_pool(name="psum", bufs=1, space="PSUM") as psum,
        ):
            # ---- input DMAs ------------------------------------------
            # sync queue: one linear DMA per x chunk. gpsimd queue:
            # g0 first (gates the first matmul, overlaps x chunk0),
            # then bias + g1..g3.
            gts = [
                wpool.tile([P, KC * OC * P], F16, tag=f"g{i}", name=f"g{i}")
                for i in range(L)
            ]
            cb_sb = wpool.tile([P, L * OC], F32, tag="cb")
            xts = {}

            def load_x(ci, eng):
                rb0, n = chunks[ci]
                t = xpool.tile([P, KC * n * 512], F16, tag=f"x{ci}",
                               name=f"x{ci}")
                off = rb0 * KC * 512 * P
                src = xt[off:off + P * KC * n * 512].rearrange(
                    "(p c) -> p c", p=P
                )
                eng.dma_start(out=t, in_=src)
                xts[ci] = t

            # scalar's HW queue fetches g0+cb while sync fetches x
            # chunk0; g1..g3 follow chunk0 on sync (the gpsimd ring is
            # too slow for anything latency-critical).
            nc.scalar.dma_start(out=gts[0], in_=gt[0])
            nc.scalar.dma_start(out=cb_sb, in_=cb)
            load_x(0, nc.sync)
            for i in range(1, L):
                nc.sync.dma_start(out=gts[i], in_=gt[i])
            for ci in range(1, NCH):
                load_x(ci, nc.sync)

            # ---- main stream: 512 back-to-back matmuls on PE ----------
            gidx = 0
            for ci, (rb0, n) in enumerate(chunks):
                for i in range(L):
                    g_i = gts[i]
                    for oc in range(OC):
                        bias = cb_sb[:, i * OC + oc:i * OC + oc + 1]
                        pbs = [
                            psum.tile([P, 512], F32, tag="d", bufs=8,
                                      name=f"p{ci}_{i}_{oc}_{rb}")
                            for rb in range(n)
                        ]
                        xc = xts[ci]
                        for k in range(KC):
                            lhsT = g_i[:, (k * OC + oc) * P:
                                       (k * OC + oc + 1) * P]
                            for rb in range(n):
                                col = (k * n + rb) * 512
                                nc.tensor.matmul(
                                    pbs[rb],
                                    lhsT,
                                    xc[:, col:col + 512],
                                    start=(k == 0),
                                    stop=(k == KC - 1),
                                )
                        ob = opool.tile([P, n * 512], F16, tag=f"ob{n}",
                                        bufs=6, name=f"ob{ci}_{i}_{oc}")
                        for rb in range(n):
                            nc.scalar.activation(
                                ob[:, rb * 512:(rb + 1) * 512], pbs[rb],
                                SIG, bias=bias,
                            )
                        dst = outT[i, oc][:, rb0 * 512:(rb0 + n) * 512]
                        eng = nc.gpsimd if gidx % 3 == 0 else nc.sync
                        eng.dma_start(out=dst, in_=ob)
                        gidx += 1

    nc.compile()
    return nc


def _prep(x, Ws, W_ff, b_ff, rows):
    """Host-side: weight composition, bias rows, x^T fp16 shards."""
    n = x.shape[0]
    c = n / (n - 1.0)
    total = x.sum(axis=0, dtype=np.float64)  # [H]
    eye = np.eye(H, dtype=np.float64)
    wfT = W_ff.astype(np.float64).T  # [H, OUT]
    M = eye.copy()
    s = np.zeros((1, H), dtype=np.float64)
    gts = np.empty((L, P, KC * OC * P), dtype=np.float16)
    cbv = np.empty((P, L * OC), dtype=np.float32)
    for i in range(L):
        WiT = Ws[i].astype(np.float64).T
        M = M @ (eye + c * WiT)
        s = s @ (eye + c * WiT) + (total[None, :] / (n - 1.0)) @ WiT
        Gi = M @ wfT                                   # [H, OUT]
        ci = b_ff.astype(np.float64) - (s @ wfT)[0]    # [OUT]
        gts[i] = (
            Gi.astype(np.float16)
            .reshape(KC, P, OC, P)
            .transpose(1, 0, 2, 3)
            .reshape(P, KC * OC * P)
        )
        cbv[:, i * OC:(i + 1) * OC] = ci.reshape(OC, P).T.astype(np.float32)

    chunks = _row_chunks(rows // 512)
    xt_maps = []
    for ccore in range(N_CORES):
        xc = x[ccore * rows:(ccore + 1) * rows]        # [rows, H]
        xtc = np.ascontiguousarray(xc.T, dtype=np.float16)  # [H, rows]
        xkc = xtc.reshape(KC, P, rows)
        flat = np.empty(KC * P * rows, dtype=np.float16)
        pos = 0
        for rb0, n in chunks:
            blk = xkc[:, :, rb0 * 512:(rb0 + n) * 512]  # [KC, P, n*512]
            sz = KC * P * n * 512
            flat[pos:pos + sz] = blk.transpose(1, 0, 2).ravel()
            pos += sz
        xt_maps.append(flat)
    return gts, cbv, xt_maps


_CACHE = {}


def kernel(input, Ws, W_ff, b_ff):
    x = np.asarray(input, dtype=np.float32)[0]  # [N, H]
    Ws = np.asarray(Ws, dtype=np.float32)
    W_ff = np.asarray(W_ff, dtype=np.float32)
    b_ff = np.asarray(b_ff, dtype=np.float32)
    n, h = x.shape
    rows = n // N_CORES

    if "nc" not in _CACHE:
        _CACHE["nc"] = build(rows=rows)
    nc = _CACHE["nc"]

    gts, cbv, xt_maps = _prep(x, Ws, W_ff, b_ff, rows)
    in_maps = [
        {"xt": xt_maps[c], "gt": gts, "cb": cbv} for c in range(N_CORES)
    ]
    res = bass_utils.run_bass_kernel_spmd(
        nc, in_maps, core_ids=list(range(N_CORES))
    )
    out = np.empty((L, n, H), dtype=np.float32)
    for c in range(N_CORES):
        o = np.asarray(res.results[c]["outT"])  # [L, OC, P, rows] f16
        out[:, c * rows:(c + 1) * rows, :] = (
            o.transpose(0, 3, 1, 2).reshape(L, rows, H).astype(np.float32)
        )
    return out


# revision 14
# speedup vs baseline: 1.0707x; 1.0707x over previous
"""Trainium2 Bass kernel for nn_Differ_Amplifier (gnn_message_passing).

Reference computation (per layer i, h0 = x [N, H]):
    represent = (N*h - colsum(h)) / (N-1)
    h = represent @ W_i.T + h
    out_i = sigmoid(h @ W_ff.T + b_ff)

Reformulation (exact algebra, validated vs fp64):
  - colsum(h) is invariant across layers (the centered "represent" sums
    to zero), so total = colsum(x), computed on the HOST from the full
    input - no collective needed at all.
  - Composing the per-layer affine maps on the host:
        h_{i+1} = h_i @ V_i - r_i,   V_i = I + c*W_i^T,  c = N/(N-1)
        M_{i+1} = M_i @ V_i,         s_{i+1} = s_i @ V_i + r_i
        out_i   = sigmoid(x @ G_i + c_i),
        G_i = M_{i+1} @ W_ff^T,      c_i = b_ff - s_{i+1} @ W_ff^T
    Four independent [rows,512]@[512,512] matmuls; the bias is a
    per-output-column constant.

Device schedule (per core, rows = 4096, everything fp16 except PSUM):
  - x is uploaded pre-transposed (x^T, fp16) so no on-device transpose.
  - Output is computed TRANSPOSED: out^T tiles [128 o-part, rows free].
    lhsT (stationary) = G blocks [128 h, 128 o], moving = x^T slices
    [128 h, 512 rows]. This makes the bias c_i[o] a per-PARTITION
    scalar, so the ACT engine applies sigmoid(z + bias) in a single op
    straight out of PSUM -> fp16 SBUF. No DVE work at all.
  - PE runs one uninterrupted stream of 512 N=512 fp16 matmuls
    (~213ns each at full clock); PSUM rotates 8 banks in two half-sets
    so ACT eviction of one half overlaps matmuls of the other.
  - DMA queues: sync=x^T in, gpsimd=weights in, vector=out^T out.
    All transfers are large and linear; host reassembles/casts fp32.
"""

import numpy as np

import concourse.bass as bass  # noqa: F401
import concourse.tile as tile
from concourse import bacc, mybir
from concourse import bass_utils

N_CORES = 8
N_TOTAL = 32768
H = 512
OUT = 512
L = 4
P = 128
KC = H // P    # 4 k-chunks of the hidden (contraction) dim
OC = OUT // P  # 4 output-column chunks
F16 = mybir.dt.float16
F32 = mybir.dt.float32
SIG = mybir.ActivationFunctionType.Sigmoid


def _row_chunks(rbt):
    """Split rbt row-blocks (512 rows each) into chunks.

    First and last chunks are single blocks (fast pipeline start, short
    tail); the middle is split into near-equal chunks of <= 4 blocks
    (one PSUM half-set each).
    """
    if rbt <= 2:
        sizes = [1] * rbt
    else:
        rem = rbt - 2
        parts = -(-rem // 4)
        base, extra = divmod(rem, parts)
        sizes = [1] + [base + (1 if j < extra else 0) for j in range(parts)] + [1]
    chunks = []
    rb = 0
    for n in sizes:
        chunks.append((rb, n))
        rb += n
    return chunks


def build(rows=N_TOTAL // N_CORES):
    """Build the SPMD kernel for one core owning `rows` rows."""
    assert rows % 512 == 0
    RBT = rows // 512
    chunks = _row_chunks(RBT)
    NCH = len(chunks)

    nc = bacc.Bacc(
        "TRN2", target_bir_lowering=False, debug=False, num_devices=N_CORES
    )
    # x^T fp16, packed chunk-major: for ci: for k: block [P, n*512]
    # raveled, so every DMA is fully linear
    xt = nc.dram_tensor("xt", [KC * P * rows], F16,
                        kind="ExternalInput").ap()
    # G blocks fp16 per layer: [P(h), (k*OC+oc)*P + m]
    gt = nc.dram_tensor("gt", [L, P, KC * OC * P], F16,
                        kind="ExternalInput").ap()
    # bias per-partition scalars: cb[p, i*OC+oc] = c_i[oc*P+p]
    cb = nc.dram_tensor("cb", [P, L * OC], F32, kind="ExternalInput").ap()
    # transposed output: [L, OC, P(o), rows]
    outT = nc.dram_tensor("outT", [L, OC, P, rows], F16,
                          kind="ExternalOutput").ap()

    with tile.TileContext(nc) as tc:
        with (
            tc.tile_pool(name="wpool", bufs=1) as wpool,
            tc.tile_pool(name="xpool", bufs=1) as xpool,
            tc.tile_pool(name="opool", bufs=1) as opool,
            tc.tile_pool(name="psum", bufs=1, space="PSUM") as psum,
        ):
            # ---- input DMAs ------------------------------------------
            # sync queue: one linear DMA per x chunk. gpsimd queue:
            # g0 first (gates the first matmul, overlaps x chunk0),
            # then bias + g1..g3.
            gts = [
                wpool.tile([P, KC * OC * P], F16, tag=f"g{i}", name=f"g{i}")
                for i in range(L)
            ]
            cb_sb = wpool.tile([P, L * OC], F32, tag="cb")
            xts = {}

            def load_x(ci, eng):
                rb0, n = chunks[ci]
                t = xpool.tile([P, KC * n * 512], F16, tag=f"x{ci}",
                               name=f"x{ci}")
                off = rb0 * KC * 512 * P
                src = xt[off:off + P * KC * n * 512].rearrange(
                    "(p c) -> p c", p=P
                )
                eng.dma_start(out=t, in_=src)
                xts[ci] = t

            # scalar's HW queue fetches g0+cb while sync fetches x
            # chunk0; g1..g3 follow chunk0 on sync (the gpsimd ring is
            # too slow for anything latency-critical).
            nc.scalar.dma_start(out=gts[0], in_=gt[0])
            nc.scalar.dma_start(out=cb_sb, in_=cb)
            load_x(0, nc.sync)
            for i in range(1, L):
                nc.sync.dma_start(out=gts[i], in_=gt[i])
            for ci in range(1, NCH):
                load_x(ci, nc.sync)

            # ---- main stream: 512 back-to-back matmuls on PE ----------
            gidx = 0
            for ci, (rb0, n) in enumerate(chunks):
                for i in range(L):
                    g_i = gts[i]
                    for oc in range(OC):
                        bias = cb_sb[:, i * OC + oc:i * OC + oc + 1]
                        pbs = [
                            psum.tile([P, 512], F32, tag="d", bufs=8,
                                      name=f"p{ci}_{i}_{oc}_{rb}")
                            for rb in range(n)
                        ]
                        xc = xts[ci]
                        for k in range(KC):
                            lhsT = g_i[:, (k * OC + oc) * P:
                                       (k * OC + oc + 1) * P]
                            for rb in range(n):
                                col = (k * n + rb) * 512
                                nc.tensor.matmul(
                                    pbs[rb],
                                    lhsT,
                                    xc[:, col:col + 512],
                                    start=(k == 0),
                                    stop=(k == KC - 1),
                                )
                        ob = opool.tile([P, n * 512], F16, tag=f"ob{n}",
                                        bufs=(16 if n == 1 else 8),
                                        name=f"ob{ci}_{i}_{oc}")
                        for rb in range(n):
                            nc.scalar.activation(
                                ob[:, rb * 512:(rb + 1) * 512], pbs[rb],
                                SIG, bias=bias,
                            )
                        dst = outT[i, oc][:, rb0 * 512:(rb0 + n) * 512]
                        last = ci == NCH - 1
                        eng = (
                            nc.gpsimd if (gidx % 3 == 0 and not last)
                            else nc.sync
                        )
                        eng.dma_start(out=dst, in_=ob)
                        gidx += 1

    nc.compile()
    return nc


def _prep(x, Ws, W_ff, b_ff, rows):
    """Host-side: weight composition, bias rows, x^T fp16 shards."""
    n = x.shape[0]
    c = n / (n - 1.0)
    total = x.sum(axis=0, dtype=np.float64)  # [H]
    eye = np.eye(H, dtype=np.float64)
    wfT = W_ff.astype(np.float64).T  # [H, OUT]
    M = eye.copy()
    s = np.zeros((1, H), dtype=np.float64)
    gts = np.empty((L, P, KC * OC * P), dtype=np.float16)
    cbv = np.empty((P, L * OC), dtype=np.float32)
    for i in range(L):
        WiT = Ws[i].astype(np.float64).T
        M = M @ (eye + c * WiT)
        s = s @ (eye + c * WiT) + (total[None, :] / (n - 1.0)) @ WiT
        Gi = M @ wfT                                   # [H, OUT]
        ci = b_ff.astype(np.float64) - (s @ wfT)[0]    # [OUT]
        gts[i] = (
            Gi.astype(np.float16)
            .reshape(KC, P, OC, P)
            .transpose(1, 0, 2, 3)
            .reshape(P, KC * OC * P)
        )
        cbv[:, i * OC:(i + 1) * OC] = ci.reshape(OC, P).T.astype(np.float32)

    chunks = _row_chunks(rows // 512)
    xt_maps = []
    for ccore in range(N_CORES):
        xc = x[ccore * rows:(ccore + 1) * rows]        # [rows, H]
        xtc = np.ascontiguousarray(xc.T, dtype=np.float16)  # [H, rows]
        xkc = xtc.reshape(KC, P, rows)
        flat = np.empty(KC * P * rows, dtype=np.float16)
        pos = 0
        for rb0, n in chunks:
            blk = xkc[:, :, rb0 * 512:(rb0 + n) * 512]  # [KC, P, n*512]
            sz = KC * P * n * 512
            flat[pos:pos + sz] = blk.transpose(1, 0, 2).ravel()
            pos += sz
        xt_maps.append(flat)
    return gts, cbv, xt_maps


_CACHE = {}


def kernel(input, Ws, W_ff, b_ff):
    x = np.asarray(input, dtype=np.float32)[0]  # [N, H]
    Ws = np.asarray(Ws, dtype=np.float32)
    W_ff = np.asarray(W_ff, dtype=np.float32)
    b_ff = np.asarray(b_ff, dtype=np.float32)
    n, h = x.shape
    rows = n // N_CORES

    if "nc" not in _CACHE:
        _CACHE["nc"] = build(rows=rows)
    nc = _CACHE["nc"]

    gts, cbv, xt_maps = _prep(x, Ws, W_ff, b_ff, rows)
    in_maps = [
        {"xt": xt_maps[c], "gt": gts, "cb": cbv} for c in range(N_CORES)
    ]
    res = bass_utils.run_bass_kernel_spmd(
        nc, in_maps, core_ids=list(range(N_CORES))
    )
    out = np.empty((L, n, H), dtype=np.float32)
    for c in range(N_CORES):
        o = np.asarray(res.results[c]["outT"])  # [L, OC, P, rows] f16
        out[:, c * rows:(c + 1) * rows, :] = (
            o.transpose(0, 3, 1, 2).reshape(L, rows, H).astype(np.float32)
        )
    return out
